# revision 22
# baseline (speedup 1.0000x reference)
"""Trainium2 Bass kernel: CNN encoder (conv1d F=8, D=128 -> K=256, valid, + bias + ReLU).

Computation: out[b, l, k] = relu(b_k[k] + sum_{f,d} x[b, l+f, d] * filt[f,d] * W[f*D+d, k])
for l in [0, L-F)  (2040 windows).

Strategy:
  - Data-parallel: 32 batches / 8 cores = 4 batches per core. Params replicated.
  - Host folds filt into W (Wp[f,d,k] = filt[f,d]*W[f*128+d,k]) and transposes x to
    d-major (xT[b, d, l]) so the contraction dim (d=128) lands on SBUF partitions
    with fully-contiguous DMA.
  - On device: for each 512-wide stripe of output positions l and each half of k,
    accumulate 8 matmuls (one per filter tap f) into one PSUM bank:
        psum[k=128p, l=512] += Wp[f,:,kh].T @ xT[:, l0+f : l0+f+512]
    using float32r (full-rate single-pass fp32 matmul; moving dim 512 >= 256).
  - Eviction fuses bias-add + ReLU in one op (bias is per-partition since k is the
    partition dim), alternating ScalarE activation / VectorE tensor_scalar.
  - Output written k-major ([b, k, l]); host transposes back to [b, l, k].
"""

import numpy as np

import concourse.bacc as bacc
import concourse.bass as bass
import concourse.tile as tile
import concourse.mybir as mybir
from concourse.bass_utils import run_bass_kernel_spmd

F32 = mybir.dt.float32
F32R = mybir.dt.float32r

N_CORES = 8
B, L, D = 32, 2048, 128
F, K = 8, 256
N_WIN = L - F            # 2040
BP = B // N_CORES        # batches per core
KH = K // 128            # k halves
# output-position stripes per batch: 3x512 + 1x504
SUPERS = [(0, 512), (512, 512), (1024, 512), (1536, N_WIN - 1536)]

# matmul input dtype: F32R = single-pass fp32 (full PE rate at N>=256), F32 = 2-pass
MM_DT = F32R


def _build_program(reps=1, loop_n=0):
    """One SPMD program for all 8 cores. reps>1 unrolls the full body (input
    DMAs + compute + output DMAs); rep r writes to output rows [r*BP, (r+1)*BP).
    loop_n>0 additionally wraps the body in a hardware For_i loop (benchmarking
    only: every loop iteration rewrites the same output region)."""
    nc = bacc.Bacc(
        "TRN2",
        target_bir_lowering=False,
        debug=False,
        num_devices=N_CORES,
    )
    xT_d = nc.declare_dram_parameter("xT", [BP, D, L], MM_DT, isOutput=False)
    wp_d = nc.declare_dram_parameter("wp", [KH, D, F, 128], MM_DT, isOutput=False)
    bias_d = nc.declare_dram_parameter("bias", [128, KH], F32, isOutput=False)
    out_d = nc.declare_dram_parameter(
        "outT", [reps * BP, KH, 128, N_WIN], F32, isOutput=True)

    # xt halves: lo covers l in [0, HALF+F), hi covers [HALF, L). Supers 0-1
    # read only lo, supers 2-3 only hi, so each matmul depends on exactly one
    # half-tile DMA (~516KB) instead of the full 1MB batch load.
    HALF = L // 2
    LO_W = HALF + F  # 1032

    # out-DMA stream points: after super si, DMA ob columns [lo, hi)
    OUT_CHUNKS = {1: (0, 1024), 2: (1024, 1536), 3: (1536, N_WIN)}

    def body(nc, tc, pools, r, warm):
        const_pool, xt_pool, psum_pool, out_pool = pools
        bias_sb = const_pool.tile([128, KH], F32, tag="bias")
        wp_sb = []
        for kh in range(KH):
            t_wp = const_pool.tile([D, F, 128], MM_DT, tag=f"wp{kh}")
            wp_sb.append(t_wp)

        xt_lo, xt_hi = [], []
        for b in range(BP):
            t_lo = xt_pool.tile([D, LO_W], MM_DT, tag="xtlo")
            t_hi = xt_pool.tile([D, L - HALF], MM_DT, tag="xthi")
            xt_lo.append(t_lo)
            xt_hi.append(t_hi)

        if warm:
            # PE HAM warm-up on junk data while the first input DMAs land.
            # Plain fp32 matmuls (4 cycles/row, ~850ns each) keep the PE busy
            # through the ~3.5us clock-gate window without f32r's
            # rounded-producer requirement.
            warm_x = const_pool.tile([D, 64], F32, tag="warmx")
            warm_ps = psum_pool.tile([128, 512], F32, tag="ps")
            nc.gpsimd.memset(warm_x[:], 0.0)
            for _ in range(16):
                nc.tensor.matmul(warm_ps[0:64, 0:64], lhsT=warm_x[:, 0:64],
                                 rhs=warm_x[:], start=True, stop=True)

        # issue order: batch-0 lo + first weight half first so compute starts ASAP
        nc.sync.dma_start(xt_lo[0][:], xT_d[0, :, 0:LO_W])
        nc.sync.dma_start(wp_sb[0][:], wp_d[0])
        nc.sync.dma_start(bias_sb[:], bias_d[:])
        nc.sync.dma_start(wp_sb[1][:], wp_d[1])
        nc.sync.dma_start(xt_hi[0][:], xT_d[0, :, HALF:L])
        for b in range(1, BP):
            nc.sync.dma_start(xt_lo[b][:], xT_d[b, :, 0:LO_W])
            nc.sync.dma_start(xt_hi[b][:], xT_d[b, :, HALF:L])

        evictor = 0
        for b in range(BP):
            for kh in range(KH):
                ob = out_pool.tile([128, N_WIN], F32, tag="ob")
                for si, (l0, ls) in enumerate(SUPERS):
                    xt = xt_lo[b] if si < 2 else xt_hi[b]
                    base = l0 if si < 2 else l0 - HALF
                    ps = psum_pool.tile([128, 512], F32, tag="ps")
                    for f in range(F):
                        nc.tensor.matmul(
                            ps[:, :ls],
                            lhsT=wp_sb[kh][:, f, :],
                            rhs=xt[:, base + f:base + f + ls],
                            start=(f == 0),
                            stop=(f == F - 1),
                        )
                    if evictor == 0:
                        nc.scalar.activation(
                            ob[:, l0:l0 + ls], ps[:, :ls],
                            mybir.ActivationFunctionType.Relu,
                            bias=bias_sb[:, kh:kh + 1], scale=1.0,
                        )
                    else:
                        nc.vector.tensor_scalar(
                            ob[:, l0:l0 + ls], ps[:, :ls],
                            scalar1=bias_sb[:, kh:kh + 1], scalar2=0.0,
                            op0=mybir.AluOpType.add, op1=mybir.AluOpType.max,
                        )
                    evictor ^= 1
                    if si in OUT_CHUNKS:
                        lo, hi = OUT_CHUNKS[si]
                        nc.sync.dma_start(out_d[r * BP + b, kh, :, lo:hi],
                                          ob[:, lo:hi])

    with tile.TileContext(nc) as tc:
        with (
            tc.tile_pool(name="const", bufs=2) as const_pool,
            tc.tile_pool(name="xt", bufs=BP) as xt_pool,
            tc.tile_pool(name="psum", bufs=6, space=bass.MemorySpace.PSUM) as psum_pool,
            tc.tile_pool(name="out", bufs=4) as out_pool,
        ):
            pools = (const_pool, xt_pool, psum_pool, out_pool)
            if loop_n > 0:
                with tc.For_i(0, loop_n, 1,
                              hint_engines=(mybir.EngineType.PE,)):
                    for r in range(reps):
                        body(nc, tc, pools, r, warm=(r == 0))
            else:
                for r in range(reps):
                    body(nc, tc, pools, r, warm=(r == 0))
    nc.compile()
    return nc


def _prep_inputs(user_batch, filt, W_k, b_k):
    user_batch = np.asarray(user_batch, dtype=np.float32)
    filt = np.asarray(filt, dtype=np.float32)
    W_k = np.asarray(W_k, dtype=np.float32)
    b_k = np.asarray(b_k, dtype=np.float32)

    wp = W_k.reshape(F, D, K) * filt[:, :, None]          # [f, d, k]
    wp_host = np.ascontiguousarray(                        # [kh, d, f, 128]
        wp.reshape(F, D, KH, 128).transpose(2, 1, 0, 3))
    bias_host = np.ascontiguousarray(b_k.reshape(KH, 128).T)  # [128, kh]
    xT = np.ascontiguousarray(user_batch.transpose(0, 2, 1))  # [b, d, l]
    return xT, wp_host, bias_host


def _run(user_batch, filt, W_k, b_k, trace=False):
    xT, wp_host, bias_host = _prep_inputs(user_batch, filt, W_k, b_k)
    nc = _build_program()
    in_maps = [
        {"xT": xT[c * BP:(c + 1) * BP], "wp": wp_host, "bias": bias_host}
        for c in range(N_CORES)
    ]
    res = run_bass_kernel_spmd(nc, in_maps, list(range(N_CORES)), trace=trace)
    outT = np.concatenate([r["outT"] for r in res.results], axis=0)  # [B, KH, 128, N_WIN]
    out = outT.reshape(B, K, N_WIN).transpose(0, 2, 1)               # [B, N_WIN, K]
    return np.ascontiguousarray(out), res


def kernel(user_batch, filt, W_k, b_k):
    out, _ = _run(user_batch, filt, W_k, b_k, trace=False)
    return out
